# revision 2
# baseline (speedup 1.0000x reference)
"""Trainium2 Bass kernel for nn_AlignmentMatrix.

score[b,i,j] = [ctx_i ; asp_j ; ctx_i*asp_j] @ w_u
            = sum_d ctx[b,i,d]*w3[d]*asp[b,j,d] + ctx[b]@w1 + asp[b]@w2

Reformulated per batch as a single matmul with host-marshalled operands:
    out[b] = ctxp[b] @ R[b]
where (D=400, Dp=512 padded)
    ctxp[b][i, 0:400] = ctx[b][i, :]   (fp16)
    ctxp[b][i, 400]   = 1.0            (bias lane)
    ctxp[b][i, 401:]  = 0
    R[b][d, j] = w3[d]*asp[b,j,d] + w1[d]   for d < 400   (folds ctx@w1)
    R[b][400, j] = asp[b,j,:] @ w2                        (folds asp@w2)
    R[b][401:, j] = 0
Both are fp16; the 54.9 GFLOP contraction runs on-device with fp32 PSUM
accumulation, host prep is O(B*L*D) elementwise marshalling.

Data-parallel across the batch dim on 8 NeuronCores (8 batches/core).
Per core the device pipeline is:
  - ctx loads: one DMA per 512-row group, partition p holds rows
    4p+r (r=0..3) so every partition line is a single 4KB contiguous
    HBM descriptor.
  - XBAR DMA transpose (sync HWDGE) turns each [128, 2048] natural
    group into 16 [128,128] transposed d-major blocks -- the PE does
    no transpose work at all.
  - main matmuls: per row-slot, 4 accumulating K=128 matmuls with
    N=512 moving (R blocks), fp32 PSUM.
  - PSUM->SBUF fp16 copies alternate scalar/vector engines; stores are
    4KB-per-partition contiguous DMAs. Host upcasts to f32.
"""

import numpy as np

import concourse.bass as bass
from concourse import bacc
import concourse.mybir as mybir
import concourse.tile as tile
from concourse.bass_utils import run_bass_kernel_spmd

F32 = mybir.dt.float32
F16 = mybir.dt.float16

B, LC, LA, D = 64, 2048, 512, 400
DP = 512          # padded contraction dim (400 data + ones lane + zeros)
NCH = 4           # K blocks of 128 covering DP
N_CORES = 8
PB = B // N_CORES  # batches per core
P = 128
RSLOT = 4          # consecutive rows per partition
GROUP = P * RSLOT  # ctx rows per group (512)


def build_kernel(pb: int = PB, lc: int = LC) -> bass.Bass:
    nc = bacc.Bacc(
        "TRN2",
        target_bir_lowering=False,
        debug=False,
        num_devices=N_CORES,
    )
    ctx_d = nc.dram_tensor("ctx", [pb, lc, DP], F16, kind="ExternalInput").ap()
    rr_d = nc.dram_tensor("rr", [pb, P, NCH, LA], F16, kind="ExternalInput").ap()
    out_d = nc.dram_tensor("out", [pb, lc, LA], F16, kind="ExternalOutput").ap()

    with tile.TileContext(nc) as tc:
        _kernel_body(tc, out_d, ctx_d, rr_d, pb, lc)
    nc.compile()
    return nc


def _kernel_body(tc, out_d, ctx_d, rr_d, pb, lc):
    nc = tc.nc
    n_groups = lc // GROUP  # ctx groups per batch

    ctx_pool = tc.alloc_tile_pool(name="ctxN", bufs=3)
    ctxT_pool = tc.alloc_tile_pool(name="ctxT", bufs=3)
    rr_pool = tc.alloc_tile_pool(name="rrt", bufs=2)
    out_pool = tc.alloc_tile_pool(name="outT", bufs=3)
    psum_o = tc.alloc_tile_pool(name="psumO", bufs=6, space="PSUM")

    copy_parity = 0
    for b in range(pb):
        # R for batch b: [128 dd, 4 c, 512 j]; 4KB/partition contiguous.
        rrt = rr_pool.tile([P, NCH * LA], F16, tag="rrt", name=f"rrt_{b}")
        nc.gpsimd.dma_start(
            out=rrt.rearrange("p (c j) -> p c j", c=NCH),
            in_=rr_d[b],
        )

        for g in range(n_groups):
            r0 = g * GROUP
            # natural-layout group: partition p holds rows 4p..4p+3
            ctxN = ctx_pool.tile([P, RSLOT * DP], F16, tag="ctxN",
                                 name=f"ctxN_{b}_{g}")
            nc.gpsimd.dma_start(
                out=ctxN.rearrange("p (r dp) -> p r dp", r=RSLOT),
                in_=ctx_d[b, r0 : r0 + GROUP, :].rearrange(
                    "(p r) dp -> p r dp", p=P
                ),
            )
            # one XBAR transpose for the whole group: block t=(4r+c) of the
            # output is ctxN[:, 128t:128t+128].T, i.e. d-major [dd, i]
            ctxT = ctxT_pool.tile([P, RSLOT * DP], F16, tag="ctxT",
                                  name=f"ctxT_{b}_{g}")
            nc.sync.dma_start(
                out=ctxT.rearrange("a (t b) -> a t b", t=RSLOT * NCH),
                in_=ctxN,
                transpose=True,
            )

            ot = out_pool.tile([P, RSLOT * LA], F16, tag="ot",
                               name=f"ot_{b}_{g}")
            for r in range(RSLOT):
                pO = psum_o.tile([P, LA], F32, tag="pO", name=f"pO_{b}_{g}_{r}")
                for c in range(NCH):
                    t = RSLOT * r + c
                    nc.tensor.matmul(
                        pO,
                        ctxT[:, t * P : (t + 1) * P],
                        rrt[:, c * LA : (c + 1) * LA],
                        start=(c == 0),
                        stop=(c == NCH - 1),
                    )
                dst = ot[:, r * LA : (r + 1) * LA]
                if copy_parity & 1:
                    nc.vector.tensor_copy(dst, pO)
                else:
                    nc.scalar.copy(dst, pO)
                copy_parity += 1

            # store: partition p -> rows 4p..4p+3, 4KB contiguous both sides
            nc.scalar.dma_start(
                out=out_d[b, r0 : r0 + GROUP, :].rearrange(
                    "(p r) j -> p r j", p=P
                ),
                in_=ot.rearrange("p (r j) -> p r j", r=RSLOT),
            )

    for p in reversed((ctx_pool, ctxT_pool, rr_pool, out_pool, psum_o)):
        p.release()


def _prep_inputs(ctx, asp, w_u):
    """Host-side marshalling: fp16 cast + padding + R formation."""
    ctx = np.asarray(ctx, dtype=np.float32)
    asp = np.asarray(asp, dtype=np.float32)
    w = np.asarray(w_u, dtype=np.float32).reshape(-1)
    w1, w2, w3 = w[:D], w[D : 2 * D], w[2 * D :]

    ctxp = np.zeros((B, LC, DP), dtype=np.float16)
    ctxp[:, :, :D] = ctx
    ctxp[:, :, D] = 1.0

    # R[b, dd, c, j]: block c rows dd -> d = 128c + dd
    scaled = (asp * w3[None, None, :] + w1[None, None, :]).transpose(0, 2, 1)
    # scaled: [B, 400 d, 512 j] f32
    rr = np.zeros((B, P, NCH, LA), dtype=np.float16)
    for c in range(3):
        rr[:, :, c, :] = scaled[:, P * c : P * (c + 1), :]
    rr[:, : D - 3 * P, 3, :] = scaled[:, 3 * P :, :]
    rr[:, D - 3 * P, 3, :] = asp @ w2  # asp_term row pairs with the ones lane
    return ctxp, rr


def kernel(batch_size=None, ctx=None, asp=None, w_u=None, **_unused):
    ctxp, rr = _prep_inputs(ctx, asp, w_u)

    nc = build_kernel()
    in_maps = [
        {
            "ctx": ctxp[i * PB : (i + 1) * PB],
            "rr": rr[i * PB : (i + 1) * PB],
        }
        for i in range(N_CORES)
    ]
    res = run_bass_kernel_spmd(
        nc, in_maps, core_ids=list(range(N_CORES)), **_RUN_KWARGS
    )
    _LAST_RESULTS.clear()
    _LAST_RESULTS.append(res)
    out = np.concatenate(
        [np.asarray(res.results[i]["out"]) for i in range(N_CORES)], axis=0
    )
    return out.astype(np.float32)


# test-harness hooks: extra kwargs for run_bass_kernel_spmd (e.g. trace=True)
# and the last BassKernelResults for profiling. Unused in grading.
_RUN_KWARGS: dict = {}
_LAST_RESULTS: list = []


# revision 4
# speedup vs baseline: 1.6872x; 1.6872x over previous
"""Trainium2 Bass kernel for nn_AlignmentMatrix.

score[b,i,j] = [ctx_i ; asp_j ; ctx_i*asp_j] @ w_u
            = sum_d ctx[b,i,d]*w3[d]*asp[b,j,d] + ctx[b]@w1 + asp[b]@w2

Reformulated per batch as a single matmul over host-marshalled operands:
    out[b] = ctxp[b] @ R[b]
with (D=400)
    ctxp[b][i, 0:400] = ctx[b][i, :]           (fp16)
    ctxp[b][i, 400]   = 1.0                     (bias lane)
    R[b][d, j] = w3[d]*asp[b,j,d] + w1[d]       (folds ctx@w1)
    R[b][400, j] = asp[b,j,:] @ w2              (folds asp@w2)
The 54.9 GFLOP contraction runs on-device with fp32 PSUM accumulation;
host prep is O(B*L*D) elementwise marshalling + layout.

Marshalling choices that shape the device kernel:
  - ctx is shipped ALREADY TRANSPOSED (d-major, [pb, 401, 2048]) so the
    device does no transposition at all (no PE transpose matmuls, no
    PSUM round-trip, no XBAR DMA).  The contraction dim sits on SBUF
    partitions exactly as the PE needs it.
  - the i-axis is permuted host-side as i' = g*512 + r*128 + p
    (i = g*512 + 4*p + r) so output partition p holds 4 CONSECUTIVE
    output rows -> every store descriptor is one 4KB contiguous line.
  - R ships as [pb, 128, 4, 512] partition-major (one 4KB descriptor
    per partition per batch).
  - contraction split: K blocks {128,128,128,17}; the last block is the
    16 tail dims + the bias lane, as a K=17 matmul (no padding traffic).

Device pipeline per batch: 4 ctx-block DMA loads (batch 0 split finer
to shorten the pipeline head), then per (group, slot): 4 accumulating
matmuls (N=512 moving R blocks) -> PSUM, PSUM->SBUF fp16 copy
alternating scalar/vector engines, 4KB-per-partition stores.  The PE
runs nothing but the 512 main matmuls.  Host upcasts fp16 -> f32.
"""

import numpy as np

import concourse.bass as bass
from concourse import bacc
import concourse.mybir as mybir
import concourse.tile as tile
from concourse.bass_utils import run_bass_kernel_spmd

F32 = mybir.dt.float32
F16 = mybir.dt.float16

B, LC, LA, D = 64, 2048, 512, 400
DX = D + 1        # contraction rows incl. the bias lane
NCH = 4           # K blocks: 128, 128, 128, 17
KLAST = DX - 3 * 128
N_CORES = 8
PB = B // N_CORES  # batches per core
P = 128
RSLOT = 4          # consecutive out rows per partition
GROUP = P * RSLOT  # out rows per group (512)


def build_kernel(pb: int = PB, lc: int = LC) -> bass.Bass:
    nc = bacc.Bacc(
        "TRN2",
        target_bir_lowering=False,
        debug=False,
        num_devices=N_CORES,
    )
    ctx_d = nc.dram_tensor("ctx", [pb, DX, lc], F16, kind="ExternalInput").ap()
    rr_d = nc.dram_tensor("rr", [pb, P, NCH, LA], F16, kind="ExternalInput").ap()
    out_d = nc.dram_tensor("out", [pb, lc, LA], F16, kind="ExternalOutput").ap()

    with tile.TileContext(nc) as tc:
        _kernel_body(tc, out_d, ctx_d, rr_d, pb, lc)
    nc.compile()
    return nc


def _kernel_body(tc, out_d, ctx_d, rr_d, pb, lc):
    nc = tc.nc
    n_groups = lc // GROUP

    ctx_pool = tc.alloc_tile_pool(name="ctxT", bufs=2)
    rr_pool = tc.alloc_tile_pool(name="rrt", bufs=2)
    out_pool = tc.alloc_tile_pool(name="outT", bufs=3)
    psum_o = tc.alloc_tile_pool(name="psumO", bufs=6, space="PSUM")

    copy_parity = 0
    for b in range(pb):
        # R for batch b: [128 dd, (c, j)]; 4KB/partition contiguous.
        rrt = rr_pool.tile([P, NCH * LA], F16, tag="rrt", name=f"rrt_{b}")
        nc.gpsimd.dma_start(
            out=rrt.rearrange("p (c j) -> p c j", c=NCH),
            in_=rr_d[b],
        )

        # ctx blocks, d-major: block c occupies cols [c*lc, (c+1)*lc);
        # block 3 fills only partitions 0..KLAST-1.
        ctxT = ctx_pool.tile([P, NCH * lc], F16, tag="ctxT", name=f"ctxT_{b}")
        if b == 0:
            # fine-grained first batch: group 0's operands land first so
            # the PE starts ~3us in instead of waiting for the full 1.6MB
            for g in range(n_groups):
                i0 = g * GROUP
                for c in range(3):
                    nc.gpsimd.dma_start(
                        out=ctxT[:, c * lc + i0 : c * lc + i0 + GROUP],
                        in_=ctx_d[b, c * P : (c + 1) * P, i0 : i0 + GROUP],
                    )
                nc.gpsimd.dma_start(
                    out=ctxT[0:KLAST, 3 * lc + i0 : 3 * lc + i0 + GROUP],
                    in_=ctx_d[b, 3 * P : DX, i0 : i0 + GROUP],
                )
        else:
            for c in range(3):
                nc.gpsimd.dma_start(
                    out=ctxT[:, c * lc : (c + 1) * lc],
                    in_=ctx_d[b, c * P : (c + 1) * P, :],
                )
            nc.gpsimd.dma_start(
                out=ctxT[0:KLAST, 3 * lc : 4 * lc],
                in_=ctx_d[b, 3 * P : DX, :],
            )

        for g in range(n_groups):
            i0 = g * GROUP
            ot = out_pool.tile([P, RSLOT * LA], F16, tag="ot",
                               name=f"ot_{b}_{g}")
            for r in range(RSLOT):
                pO = psum_o.tile([P, LA], F32, tag="pO", name=f"pO_{b}_{g}_{r}")
                for c in range(NCH):
                    kc = P if c < 3 else KLAST
                    col = c * lc + i0 + r * P
                    nc.tensor.matmul(
                        pO,
                        ctxT[0:kc, col : col + P],
                        rrt[0:kc, c * LA : (c + 1) * LA],
                        start=(c == 0),
                        stop=(c == NCH - 1),
                    )
                dst = ot[:, r * LA : (r + 1) * LA]
                if copy_parity & 1:
                    nc.vector.tensor_copy(dst, pO)
                else:
                    nc.scalar.copy(dst, pO)
                copy_parity += 1

            # store: partition p -> rows i0 + 4p + r, 4KB contiguous
            nc.sync.dma_start(
                out=out_d[b, i0 : i0 + GROUP, :].rearrange(
                    "(p r) j -> p r j", p=P
                ),
                in_=ot.rearrange("p (r j) -> p r j", r=RSLOT),
            )

    for p in reversed((ctx_pool, rr_pool, out_pool, psum_o)):
        p.release()


def _prep_inputs(ctx, asp, w_u):
    """Host-side marshalling: fp16 cast, transpose/permute, R formation."""
    ctx = np.asarray(ctx, dtype=np.float32)
    asp = np.asarray(asp, dtype=np.float32)
    w = np.asarray(w_u, dtype=np.float32).reshape(-1)
    w1, w2, w3 = w[:D], w[D : 2 * D], w[2 * D :]

    # ctxT with i' = g*512 + r*128 + p  <->  i = g*512 + 4p + r
    ctxp = np.empty((B, DX, LC), dtype=np.float16)
    # [B, i, d] -> [B, d, g, p, r] -> [B, d, g, r, p]
    ctxp[:, :D, :] = (
        np.transpose(ctx.reshape(B, LC // GROUP, P, RSLOT, D), (0, 4, 1, 3, 2))
        .reshape(B, D, LC)
        .astype(np.float16)
    )
    ctxp[:, D, :] = 1.0

    # R[b, dd, c, j]: block c rows dd -> d = 128c + dd; block 3 holds the
    # 16 tail dims + the asp_term row, rest zero (never read: K=17 matmul)
    scaled = (asp * w3[None, None, :] + w1[None, None, :]).transpose(0, 2, 1)
    rr = np.zeros((B, P, NCH, LA), dtype=np.float16)
    for c in range(3):
        rr[:, :, c, :] = scaled[:, P * c : P * (c + 1), :]
    rr[:, : D - 3 * P, 3, :] = scaled[:, 3 * P :, :]
    rr[:, D - 3 * P, 3, :] = asp @ w2
    return ctxp, rr


def kernel(batch_size=None, ctx=None, asp=None, w_u=None, **_unused):
    ctxp, rr = _prep_inputs(ctx, asp, w_u)

    nc = build_kernel()
    in_maps = [
        {
            "ctx": ctxp[i * PB : (i + 1) * PB],
            "rr": rr[i * PB : (i + 1) * PB],
        }
        for i in range(N_CORES)
    ]
    res = run_bass_kernel_spmd(
        nc, in_maps, core_ids=list(range(N_CORES)), **_RUN_KWARGS
    )
    _LAST_RESULTS.clear()
    _LAST_RESULTS.append(res)
    # stores write natural row order (partition p, slot r -> row 4p+r)
    out = np.concatenate(
        [np.asarray(res.results[i]["out"]) for i in range(N_CORES)], axis=0
    )
    return out.astype(np.float32)


# test-harness hooks: extra kwargs for run_bass_kernel_spmd (e.g. trace=True)
# and the last BassKernelResults for profiling. Unused in grading.
_RUN_KWARGS: dict = {}
_LAST_RESULTS: list = []


# revision 8
# speedup vs baseline: 1.9624x; 1.1631x over previous
"""Trainium2 Bass kernel for nn_AlignmentMatrix.

score[b,i,j] = [ctx_i ; asp_j ; ctx_i*asp_j] @ w_u
            = sum_d ctx[b,i,d]*w3[d]*asp[b,j,d] + ctx[b]@w1 + asp[b]@w2

Reformulated per batch as a single matmul over host-marshalled operands:
    out[b] = ctxp[b] @ R[b]
with (D=400)
    ctxp[b][i, 0:400] = ctx[b][i, :]           (fp16)
    ctxp[b][i, 400]   = 1.0                     (bias lane)
    R[b][d, j] = w3[d]*asp[b,j,d] + w1[d]       (folds ctx@w1)
    R[b][400, j] = asp[b,j,:] @ w2              (folds asp@w2)
The 54.9 GFLOP contraction runs on-device with fp32 PSUM accumulation;
host prep is O(B*L*D) elementwise marshalling + layout.

Marshalling choices that shape the device kernel:
  - ctx is shipped ALREADY TRANSPOSED (d-major, [pb, 401, 2048]) so the
    device does no transposition at all (no PE transpose matmuls, no
    PSUM round-trip, no XBAR DMA).  The contraction dim sits on SBUF
    partitions exactly as the PE needs it.
  - the i-axis is permuted host-side as i' = g*512 + r*128 + p
    (i = g*512 + 4*p + r) so output partition p holds 4 CONSECUTIVE
    output rows -> every store descriptor is one 4KB contiguous line.
  - R ships as [pb, 128, 4, 512] partition-major (one 4KB descriptor
    per partition per batch).
  - contraction split: K blocks {128,128,128,17}; the last block is the
    16 tail dims + the bias lane, as a K=17 matmul (no padding traffic).

Device pipeline per batch: 4 ctx-block DMA loads (batch 0 split finer
to shorten the pipeline head), then per (group, slot): 4 accumulating
matmuls (N=512 moving R blocks) -> PSUM, PSUM->SBUF fp16 copy
alternating scalar/vector engines, 4KB-per-partition stores.  The PE
runs nothing but the 512 main matmuls.  Host upcasts fp16 -> f32.
"""

import numpy as np

import concourse.bass as bass
from concourse import bacc
import concourse.mybir as mybir
import concourse.tile as tile
from concourse.bass_utils import run_bass_kernel_spmd

F32 = mybir.dt.float32
F16 = mybir.dt.float16

B, LC, LA, D = 64, 2048, 512, 400
DX = 512          # contraction rows: 400 data + bias lane + zero pad
NCH = 4           # K blocks of 128 (block 3 = tail dims + bias + zeros;
                  # full K keeps Fast Weight Load enabled on every matmul)
N_CORES = 8
PB = B // N_CORES  # batches per core
P = 128
RSLOT = 4          # consecutive out rows per partition
GROUP = P * RSLOT  # out rows per group (512)


def build_kernel(pb: int = PB, lc: int = LC) -> bass.Bass:
    nc = bacc.Bacc(
        "TRN2",
        target_bir_lowering=False,
        debug=False,
        num_devices=N_CORES,
    )
    ctx_d = nc.dram_tensor("ctx", [pb, DX, lc], F16, kind="ExternalInput").ap()
    rr_d = nc.dram_tensor("rr", [pb, P, NCH, LA], F16, kind="ExternalInput").ap()
    out_d = nc.dram_tensor("out", [pb, lc, LA], F16, kind="ExternalOutput").ap()

    with tile.TileContext(nc) as tc:
        _kernel_body(tc, out_d, ctx_d, rr_d, pb, lc)
    nc.compile()
    return nc


def _kernel_body(tc, out_d, ctx_d, rr_d, pb, lc):
    nc = tc.nc
    n_groups = lc // GROUP

    ctx_pool = tc.alloc_tile_pool(name="ctxT", bufs=2)
    rr_pool = tc.alloc_tile_pool(name="rrt", bufs=2)
    out_pool = tc.alloc_tile_pool(name="outT", bufs=3)
    psum_o = tc.alloc_tile_pool(name="psumO", bufs=6, space="PSUM")

    copy_parity = 0
    for b in range(pb):
        # R for batch b: [128 dd, (c, j)]; 4KB/partition contiguous.
        rrt = rr_pool.tile([P, NCH * LA], F16, tag="rrt", name=f"rrt_{b}")
        nc.gpsimd.dma_start(
            out=rrt.rearrange("p (c j) -> p c j", c=NCH),
            in_=rr_d[b],
        )

        # ctx blocks, d-major: block c occupies cols [c*lc, (c+1)*lc)
        ctxT = ctx_pool.tile([P, NCH * lc], F16, tag="ctxT", name=f"ctxT_{b}")
        if b == 0:
            # fine-grained first batch: group 0's operands land first so
            # the PE starts ~3us in instead of waiting for the full 2MB
            for g in range(n_groups):
                i0 = g * GROUP
                for c in range(NCH):
                    nc.gpsimd.dma_start(
                        out=ctxT[:, c * lc + i0 : c * lc + i0 + GROUP],
                        in_=ctx_d[b, c * P : (c + 1) * P, i0 : i0 + GROUP],
                    )
        else:
            for c in range(NCH):
                nc.gpsimd.dma_start(
                    out=ctxT[:, c * lc : (c + 1) * lc],
                    in_=ctx_d[b, c * P : (c + 1) * P, :],
                )

        for g in range(n_groups):
            i0 = g * GROUP
            ot = out_pool.tile([P, RSLOT * LA], F16, tag="ot",
                               name=f"ot_{b}_{g}")
            for r in range(RSLOT):
                pO = psum_o.tile([P, LA], F32, tag="pO", name=f"pO_{b}_{g}_{r}")
                for c in range(NCH):
                    col = c * lc + i0 + r * P
                    nc.tensor.matmul(
                        pO,
                        ctxT[:, col : col + P],
                        rrt[:, c * LA : (c + 1) * LA],
                        start=(c == 0),
                        stop=(c == NCH - 1),
                    )
                dst = ot[:, r * LA : (r + 1) * LA]
                if copy_parity & 1:
                    nc.vector.tensor_copy(dst, pO)
                else:
                    nc.scalar.copy(dst, pO)
                copy_parity += 1

            # store: partition p -> rows i0 + 4p + r, 4KB contiguous
            nc.sync.dma_start(
                out=out_d[b, i0 : i0 + GROUP, :].rearrange(
                    "(p r) j -> p r j", p=P
                ),
                in_=ot.rearrange("p (r j) -> p r j", r=RSLOT),
            )

    for p in reversed((ctx_pool, rr_pool, out_pool, psum_o)):
        p.release()


def _prep_inputs(ctx, asp, w_u):
    """Host-side marshalling: fp16 cast, transpose/permute, R formation."""
    ctx = np.asarray(ctx, dtype=np.float32)
    asp = np.asarray(asp, dtype=np.float32)
    w = np.asarray(w_u, dtype=np.float32).reshape(-1)
    w1, w2, w3 = w[:D], w[D : 2 * D], w[2 * D :]

    # ctxT with i' = g*512 + r*128 + p  <->  i = g*512 + 4p + r
    ctxp = np.zeros((B, DX, LC), dtype=np.float16)
    # [B, i, d] -> [B, d, g, p, r] -> [B, d, g, r, p]
    ctxp[:, :D, :] = (
        np.transpose(ctx.reshape(B, LC // GROUP, P, RSLOT, D), (0, 4, 1, 3, 2))
        .reshape(B, D, LC)
        .astype(np.float16)
    )
    ctxp[:, D, :] = 1.0  # bias lane; rows D+1.. stay zero

    # R[b, dd, c, j]: block c rows dd -> d = 128c + dd; block 3 holds the
    # 16 tail dims + the asp_term row, rest zero (never read: K=17 matmul)
    scaled = (asp * w3[None, None, :] + w1[None, None, :]).transpose(0, 2, 1)
    rr = np.zeros((B, P, NCH, LA), dtype=np.float16)
    for c in range(3):
        rr[:, :, c, :] = scaled[:, P * c : P * (c + 1), :]
    rr[:, : D - 3 * P, 3, :] = scaled[:, 3 * P :, :]
    rr[:, D - 3 * P, 3, :] = asp @ w2
    return ctxp, rr


def kernel(batch_size=None, ctx=None, asp=None, w_u=None, **_unused):
    ctxp, rr = _prep_inputs(ctx, asp, w_u)

    nc = build_kernel()
    in_maps = [
        {
            "ctx": ctxp[i * PB : (i + 1) * PB],
            "rr": rr[i * PB : (i + 1) * PB],
        }
        for i in range(N_CORES)
    ]
    res = run_bass_kernel_spmd(
        nc, in_maps, core_ids=list(range(N_CORES)), **_RUN_KWARGS
    )
    _LAST_RESULTS.clear()
    _LAST_RESULTS.append(res)
    # stores write natural row order (partition p, slot r -> row 4p+r)
    out = np.concatenate(
        [np.asarray(res.results[i]["out"]) for i in range(N_CORES)], axis=0
    )
    return out.astype(np.float32)


# test-harness hooks: extra kwargs for run_bass_kernel_spmd (e.g. trace=True)
# and the last BassKernelResults for profiling. Unused in grading.
_RUN_KWARGS: dict = {}
_LAST_RESULTS: list = []


# revision 9
# speedup vs baseline: 2.1997x; 1.1209x over previous
"""Trainium2 Bass kernel for nn_AlignmentMatrix.

score[b,i,j] = [ctx_i ; asp_j ; ctx_i*asp_j] @ w_u
            = sum_d ctx[b,i,d]*w3[d]*asp[b,j,d] + ctx[b]@w1 + asp[b]@w2

Reformulated per batch as a single matmul over host-marshalled operands:
    out[b] = ctxp[b] @ R[b]
with (D=400)
    ctxp[b][i, 0:400] = ctx[b][i, :]           (fp16)
    ctxp[b][i, 400]   = 1.0                     (bias lane)
    R[b][d, j] = w3[d]*asp[b,j,d] + w1[d]       (folds ctx@w1)
    R[b][400, j] = asp[b,j,:] @ w2              (folds asp@w2)
The 54.9 GFLOP contraction runs on-device with fp32 PSUM accumulation;
host prep is O(B*L*D) elementwise marshalling + layout.

Marshalling / kernel-structure choices:
  - ctx ships ALREADY TRANSPOSED (d-major) so the device does no
    transposition at all: 3 full K=128 blocks [pb, 384, 2048].
  - the contraction tail (16 dims + bias lane = K=17) ships separately,
    replicated at partition offsets {0,32,64,96}: the four row-slots of
    an output group run their tail matmuls CONCURRENTLY in one array
    pass via tile_position row tiling (saves ~3/16 of PE time vs
    padding the tail to K=128).
  - the i-axis is permuted host-side as i' = g*512 + r*128 + p
    (i = g*512 + 4p + r) so output partition p holds 4 CONSECUTIVE
    output rows -> every store descriptor is one 4KB contiguous line.
  - R ships as [pb, 128, 4, 512] partition-major (one 4KB descriptor
    per partition per batch); block 3 rows carry the tail R slices
    replicated at the same partition offsets.

Device pipeline per batch: ctx-block DMA loads (batch 0 split finer to
shorten the pipeline head), then per group: 12 full matmuls + the
4-way tail bundle -> PSUM, PSUM->SBUF fp16 copies alternating
scalar/vector engines, 4KB-per-partition stores.  The PE runs nothing
but main matmuls.  Host upcasts fp16 -> f32.
"""

import numpy as np

import concourse.bass as bass
from concourse import bacc
import concourse.mybir as mybir
import concourse.tile as tile
from concourse.bass_utils import run_bass_kernel_spmd

F32 = mybir.dt.float32
F16 = mybir.dt.float16

B, LC, LA, D = 64, 2048, 512, 400
NCH = 3           # full K=128 blocks; tail handled by the bundle
KT = 17           # tail rows: 16 data dims + bias lane
N_CORES = 8
PB = B // N_CORES  # batches per core
P = 128
RSLOT = 4          # consecutive out rows per partition
GROUP = P * RSLOT  # out rows per group (512)
NG = LC // GROUP   # groups per batch


def build_kernel(pb: int = PB, lc: int = LC) -> bass.Bass:
    nc = bacc.Bacc(
        "TRN2",
        target_bir_lowering=False,
        debug=False,
        num_devices=N_CORES,
    )
    ctx_d = nc.dram_tensor("ctx", [pb, NCH * P, lc], F16, kind="ExternalInput").ap()
    tl_d = nc.dram_tensor("tl", [pb, P, NG, P], F16, kind="ExternalInput").ap()
    rr_d = nc.dram_tensor("rr", [pb, P, NCH + 1, LA], F16, kind="ExternalInput").ap()
    out_d = nc.dram_tensor("out", [pb, lc, LA], F16, kind="ExternalOutput").ap()

    with tile.TileContext(nc) as tc:
        _kernel_body(tc, out_d, ctx_d, tl_d, rr_d, pb, lc)
    nc.compile()
    return nc


def _kernel_body(tc, out_d, ctx_d, tl_d, rr_d, pb, lc):
    nc = tc.nc

    ctx_pool = tc.alloc_tile_pool(name="ctxT", bufs=2)
    tl_pool = tc.alloc_tile_pool(name="tl", bufs=2)
    rr_pool = tc.alloc_tile_pool(name="rrt", bufs=2)
    out_pool = tc.alloc_tile_pool(name="outT", bufs=3)
    psum_o = tc.alloc_tile_pool(name="psumO", bufs=8, space="PSUM")

    copy_parity = 0
    for b in range(pb):
        # R for batch b: [128 dd, (c, j)]; 4KB/partition contiguous.
        rrt = rr_pool.tile([P, (NCH + 1) * LA], F16, tag="rrt", name=f"rrt_{b}")
        nc.gpsimd.dma_start(
            out=rrt.rearrange("p (c j) -> p c j", c=NCH + 1),
            in_=rr_d[b],
        )
        # tails: [128, (g, p)]; 1KB/partition contiguous
        tlt = tl_pool.tile([P, NG * P], F16, tag="tl", name=f"tl_{b}")
        nc.gpsimd.dma_start(
            out=tlt.rearrange("p (g q) -> p g q", g=NG),
            in_=tl_d[b],
        )

        # ctx blocks, d-major: block c occupies cols [c*lc, (c+1)*lc)
        ctxT = ctx_pool.tile([P, NCH * lc], F16, tag="ctxT", name=f"ctxT_{b}")
        if b == 0:
            # fine-grained first batch: group 0's operands land first so
            # the PE starts ~3us in instead of waiting for the full 1.5MB
            for g in range(NG):
                i0 = g * GROUP
                for c in range(NCH):
                    nc.gpsimd.dma_start(
                        out=ctxT[:, c * lc + i0 : c * lc + i0 + GROUP],
                        in_=ctx_d[b, c * P : (c + 1) * P, i0 : i0 + GROUP],
                    )
        else:
            for c in range(NCH):
                nc.gpsimd.dma_start(
                    out=ctxT[:, c * lc : (c + 1) * lc],
                    in_=ctx_d[b, c * P : (c + 1) * P, :],
                )

        for g in range(NG):
            i0 = g * GROUP
            ot = out_pool.tile([P, RSLOT * LA], F16, tag="ot",
                               name=f"ot_{b}_{g}")
            psl = []
            for r in range(RSLOT):
                pO = psum_o.tile([P, LA], F32, tag="pO", name=f"pO_{b}_{g}_{r}")
                psl.append(pO)
                for c in range(NCH):
                    col = c * lc + i0 + r * P
                    nc.tensor.matmul(
                        pO,
                        ctxT[:, col : col + P],
                        rrt[:, c * LA : (c + 1) * LA],
                        start=(c == 0),
                        stop=False,
                    )
            # concurrent tail bundle: slot r's K=17 matmul in array rows
            # [32r, 32r+17) -- all four share one streaming pass
            for r in range(RSLOT):
                nc.tensor.matmul(
                    psl[r],
                    tlt[32 * r : 32 * r + KT, g * P : (g + 1) * P],
                    rrt[32 * r : 32 * r + KT, NCH * LA : (NCH + 1) * LA],
                    start=False,
                    stop=True,
                    tile_position=(32 * r, 0),
                )
            for r in range(RSLOT):
                dst = ot[:, r * LA : (r + 1) * LA]
                if copy_parity & 1:
                    nc.vector.tensor_copy(dst, psl[r])
                else:
                    nc.scalar.copy(dst, psl[r])
                copy_parity += 1

            # store: partition p -> rows i0 + 4p + r, 4KB contiguous
            nc.sync.dma_start(
                out=out_d[b, i0 : i0 + GROUP, :].rearrange(
                    "(p r) j -> p r j", p=P
                ),
                in_=ot.rearrange("p (r j) -> p r j", r=RSLOT),
            )

    for p in reversed((ctx_pool, tl_pool, rr_pool, out_pool, psum_o)):
        p.release()


def _prep_inputs(ctx, asp, w_u):
    """Host-side marshalling: fp16 cast, transpose/permute, R formation."""
    ctx = np.asarray(ctx, dtype=np.float32)
    asp = np.asarray(asp, dtype=np.float32)
    w = np.asarray(w_u, dtype=np.float32).reshape(-1)
    w1, w2, w3 = w[:D], w[D : 2 * D], w[2 * D :]

    # ctxT (first 384 dims) with i' = g*512 + r*128 + p <-> i = g*512+4p+r
    # [B, i, d] -> [B, d, g, p, r] -> [B, d, g, r, p]
    cr = ctx.reshape(B, NG, P, RSLOT, D)
    ctxp = (
        np.transpose(cr[..., : NCH * P], (0, 4, 1, 3, 2))
        .reshape(B, NCH * P, LC)
        .astype(np.float16)
    )

    # tails: [B, 128, g, p]; partition 32r+t holds tail dim t (t<16) or
    # the bias lane (t=16) for slot r; column (g, p) is out row g*512+4p+r
    tails = np.zeros((B, P, NG, P), dtype=np.float16)
    tail_d = np.transpose(cr[..., NCH * P :], (0, 4, 1, 2, 3))  # [B,16,g,p,r]
    for r in range(RSLOT):
        tails[:, 32 * r : 32 * r + 16, :, :] = tail_d[..., r]
        tails[:, 32 * r + 16, :, :] = 1.0

    # R[b, dd, c, j]: blocks 0..2 rows dd -> d = 128c + dd; block 3 rows
    # 32r+t -> tail slice (replicated for each slot offset)
    scaled = (asp * w3[None, None, :] + w1[None, None, :]).transpose(0, 2, 1)
    at = asp @ w2
    rr = np.zeros((B, P, NCH + 1, LA), dtype=np.float16)
    for c in range(NCH):
        rr[:, :, c, :] = scaled[:, P * c : P * (c + 1), :]
    for r in range(RSLOT):
        rr[:, 32 * r : 32 * r + 16, NCH, :] = scaled[:, NCH * P :, :]
        rr[:, 32 * r + 16, NCH, :] = at
    return ctxp, tails, rr


def kernel(batch_size=None, ctx=None, asp=None, w_u=None, **_unused):
    ctxp, tails, rr = _prep_inputs(ctx, asp, w_u)

    nc = build_kernel()
    in_maps = [
        {
            "ctx": ctxp[i * PB : (i + 1) * PB],
            "tl": tails[i * PB : (i + 1) * PB],
            "rr": rr[i * PB : (i + 1) * PB],
        }
        for i in range(N_CORES)
    ]
    res = run_bass_kernel_spmd(
        nc, in_maps, core_ids=list(range(N_CORES)), **_RUN_KWARGS
    )
    _LAST_RESULTS.clear()
    _LAST_RESULTS.append(res)
    # stores write natural row order (partition p, slot r -> row 4p+r)
    out = np.concatenate(
        [np.asarray(res.results[i]["out"]) for i in range(N_CORES)], axis=0
    )
    return out.astype(np.float32)


# test-harness hooks: extra kwargs for run_bass_kernel_spmd (e.g. trace=True)
# and the last BassKernelResults for profiling. Unused in grading.
_RUN_KWARGS: dict = {}
_LAST_RESULTS: list = []
